# revision 1
# baseline (speedup 1.0000x reference)
"""Trainium2 Bass kernel for the diag-conv problem.

Math (full problem, NET_SUM=512, K=512):
    P[i,r,c]  = X[i,r,c] * W[c,r]                (elementwise vs W^T)
    d1[i,r]   = sum_c P[i,r,c]                   (row sums)
    d2[i,c]   = sum_r P[i,r,c]                   (col sums)
    d         = d1 + d2
    out[i,r,c] = relu(0.1*(d[i,r] + d[i,c]))

Sharding: data-parallel over the batch axis i across 8 cores (64 each).
W^T (pre-scaled by 0.1) is replicated; all math on-device is fp32.

Per-core engine mapping (per batch element):
    DMA : batch elements stream in/out PAIR at a time via HWDGE (nc.sync)
    DVE : scalar_tensor_tensor -> P (SBUF) + d1 per-partition (fused pass)
    PE  : ones-matmuls reduce P over partitions -> d2 row in PSUM;
          4 transposes of the d1 accumulator -> d1 row in PSUM;
          ones^T (x) G -> B[p,f] = G[f] broadcast; 4 tiny k=1 matmuls ->
          G chunks as per-partition bias columns
    ACT : d1-row PSUM->SBUF copy; relu(B + bias_chunk) -> output tile
"""

import numpy as np

N_CORES = 8
NET_SUM = 512
K = 512
NB = NET_SUM // N_CORES  # 64 batches per core
NT = 4                   # 512 rows = 4 groups of 128 partitions
P_DIM = 128

# dtype of the stored P product / d2 column-sum matmul inputs
D2_DT = "float32"
# replay the whole batch loop this many times inside one NEFF (timing only)
REPEAT = 1
# batch elements moved per DMA (bigger -> fewer, larger transfers)
PAIR = 4
# row->partition mapping: False = partition p holds rows {p, 128+p, ...}
# (2KB contiguous runs per partition); True = partition p holds rows
# {4p..4p+3} (8KB contiguous runs per partition per batch element)
INTERLEAVE = False
# timing experiment: skip all compute, only DMA in/out
DMA_ONLY = False
# issue output DMAs from the ACT HWDGE ring (SP ring handles loads)
SPLIT_RINGS = False
# override xp/op pool depth (None = default 3)
BUFS = None

_CACHE = {}


def build(n_batch=NB, loop_n=1):
    from contextlib import nullcontext

    import concourse.mybir as mybir
    import concourse.tile as tile
    from concourse import bacc
    from concourse.masks import make_identity

    f32 = mybir.dt.float32
    d2_dt = getattr(mybir.dt, D2_DT)

    nc = bacc.Bacc("TRN2", target_bir_lowering=False, debug=False)

    x_dram = nc.dram_tensor("x4", [n_batch, NET_SUM, K], f32, kind="ExternalInput")
    wt_dram = nc.dram_tensor("wt", [P_DIM, NT, K], f32, kind="ExternalInput")
    out_dram = nc.dram_tensor(
        "out4", [n_batch, NET_SUM, K], f32, kind="ExternalOutput"
    )

    pair = PAIR
    assert n_batch % pair == 0
    big_bufs = (BUFS or 3) if pair <= 2 else 2

    if INTERLEAVE:
        # partition p <- rows 4p..4p+3 : contiguous 8KB per partition
        dram_pat = "b (p q) f -> p b q f"
        dram_kw = {"p": P_DIM}
        # row r = 4p + q ; row-position view of a [1, 512] row AP
        row_pat, row_kw = "o (p q) -> o q p", {"q": NT}
    else:
        dram_pat = "b (q p) f -> p b q f"
        dram_kw = {"q": NT}
        row_pat, row_kw = "o (q p) -> o q p", {"p": P_DIM}

    with tile.TileContext(nc) as tc:
        with (
            tc.tile_pool(name="const", bufs=1) as const_pool,
            tc.tile_pool(name="xp", bufs=big_bufs) as xp,
            tc.tile_pool(name="pp", bufs=(BUFS or 3)) as pp,
            tc.tile_pool(name="op", bufs=big_bufs) as op,
            tc.tile_pool(name="small", bufs=4) as small,
            tc.tile_pool(name="gps", bufs=2, space="PSUM") as gps,
            tc.tile_pool(name="dps", bufs=2, space="PSUM") as dps,
            tc.tile_pool(name="bps", bufs=2, space="PSUM") as bps,
            tc.tile_pool(name="cps", bufs=2, space="PSUM") as cps,
        ):
            wt = const_pool.tile([P_DIM, NT, K], f32)
            nc.sync.dma_start(wt[:], wt_dram[:])

            identity = const_pool.tile([P_DIM, P_DIM], f32)
            make_identity(nc, identity[:])

            ones_col = const_pool.tile([P_DIM, 1], d2_dt)
            nc.vector.memset(ones_col[:], 1.0)
            ones_row = const_pool.tile([1, P_DIM], f32)
            nc.vector.memset(ones_row[:], 1.0)
            one11 = const_pool.tile([1, 1], f32)
            nc.vector.memset(one11[:], 1.0)

            loop_ctx = tc.For_i(0, loop_n, 1) if loop_n > 1 else nullcontext()
            with loop_ctx:
                for ip in [
                    i for _ in range(REPEAT) for i in range(n_batch // pair)
                ]:
                    # one DMA moves `pair` batch elements (pair MB)
                    xpair = xp.tile([P_DIM, pair, NT, K], f32)
                    nc.sync.dma_start(
                        xpair[:],
                        x_dram[:][ip * pair : (ip + 1) * pair].rearrange(
                            dram_pat, **dram_kw
                        ),
                    )
                    out_eng = nc.scalar if SPLIT_RINGS else nc.sync
                    if DMA_ONLY:
                        out_eng.dma_start(
                            out_dram[:][ip * pair : (ip + 1) * pair].rearrange(
                                dram_pat, **dram_kw
                            ),
                            xpair[:],
                        )
                        continue
                    opair = op.tile([P_DIM, pair, NT, K], f32)
                    for j in range(pair):
                        x = xpair[:, j]
                        o = opair[:, j]
                        # P = x * wt ; d1 per-partition sums (fused DVE pass;
                        # tensor_tensor_reduce crashes the DVE on this HW)
                        p = pp.tile([P_DIM, NT, K], d2_dt)
                        d1 = small.tile([P_DIM, NT], f32, tag="d1")
                        for t in range(NT):
                            nc.vector.scalar_tensor_tensor(
                                out=p[:, t, :],
                                in0=x[:, t, :],
                                scalar=1.0,
                                in1=wt[:, t, :],
                                op0=mybir.AluOpType.mult,
                                op1=mybir.AluOpType.mult,
                                accum_out=d1[:, t : t + 1],
                            )

                        # d2 row [1,512] (column sums): clean 4-matmul
                        # PSUM accumulation group
                        psum_g = gps.tile([1, K], f32)
                        for t in range(NT):
                            nc.tensor.matmul(
                                psum_g[:, :],
                                ones_col[:],
                                p[:, t, :],
                                start=(t == 0),
                                stop=(t == NT - 1),
                            )
                        # d1 row [1,512]: 4 independent PE transposes of the
                        # d1 accumulator columns into row positions
                        psum_d1r = dps.tile([1, K], f32)
                        d1r_rows = psum_d1r[:].rearrange(row_pat, **row_kw)
                        for t in range(NT):
                            nc.tensor.matmul(
                                d1r_rows[:, t, :],
                                d1[:, t : t + 1],
                                identity[:],
                                is_transpose=True,
                                start=True,
                                stop=True,
                                skip_group_check=True,
                            )
                        # G = d1 + d2 (DVE reads only one PSUM operand, so
                        # the d1 row goes through SBUF via an ACT copy)
                        d1row = small.tile([1, K], f32, tag="d1row")
                        nc.scalar.copy(d1row[:], psum_d1r[:])
                        g = small.tile([1, K], f32, tag="g")
                        nc.vector.tensor_tensor(
                            out=g[:],
                            in0=psum_g[:],
                            in1=d1row[:],
                            op=mybir.AluOpType.add,
                        )

                        # B[p,f] = G[f] for all p (rank-1 broadcast matmul)
                        psum_b = bps.tile([P_DIM, K], f32)
                        nc.tensor.matmul(
                            psum_b[:],
                            ones_row[:],
                            g[:],
                            start=True,
                            stop=True,
                        )

                        # bias columns: gcol[p, t] = G[row(p, t)]
                        g_rows = g[:].rearrange(row_pat, **row_kw)
                        psum_gc = cps.tile([P_DIM, NT], f32)
                        for t in range(NT):
                            nc.tensor.matmul(
                                psum_gc[:, t : t + 1],
                                g_rows[:, t, :],
                                one11[:],
                                start=True,
                                stop=True,
                            )
                        gcol = small.tile([P_DIM, NT], f32, tag="gcol")
                        nc.scalar.copy(gcol[:], psum_gc[:])

                        # o[p, t, f] = relu(B[p,f] + gcol[p,t])
                        for t in range(NT):
                            nc.scalar.activation(
                                out=o[:, t, :],
                                in_=psum_b[:],
                                func=mybir.ActivationFunctionType.Relu,
                                bias=gcol[:, t : t + 1],
                                scale=1.0,
                            )

                    out_eng.dma_start(
                        out_dram[:][ip * pair : (ip + 1) * pair].rearrange(
                            dram_pat, **dram_kw
                        ),
                        opair[:],
                    )

    nc.compile()
    return nc


def _prep_host(input_feature, kernel):
    x = np.ascontiguousarray(np.asarray(input_feature, dtype=np.float32))
    w = np.asarray(kernel, dtype=np.float32)
    a = (0.1 * w.T).astype(np.float32)  # a[r, j] = 0.1 * w[j, r]
    if INTERLEAVE:
        wt = np.ascontiguousarray(a.reshape(P_DIM, NT, K))
    else:
        wt = np.ascontiguousarray(a.reshape(NT, P_DIM, K).transpose(1, 0, 2))
    x4 = x.reshape(N_CORES, NB, NET_SUM, K)
    return x4, wt


TRACE = False
LAST_RESULTS = None


def kernel(input_feature, kernel):
    global LAST_RESULTS
    from concourse.bass_utils import run_bass_kernel_spmd

    x4, wt = _prep_host(input_feature, kernel)

    if "nc" not in _CACHE:
        _CACHE["nc"] = build()
    nc = _CACHE["nc"]

    in_maps = [{"x4": np.ascontiguousarray(x4[c]), "wt": wt} for c in range(N_CORES)]
    res = run_bass_kernel_spmd(nc, in_maps, core_ids=list(range(N_CORES)), trace=TRACE)
    LAST_RESULTS = res
    out = np.concatenate([r["out4"] for r in res.results], axis=0)
    return out



# revision 4
# speedup vs baseline: 1.2425x; 1.2425x over previous
"""Trainium2 Bass kernel for the diag-conv problem (fp16 I/O).

Math (full problem, NET_SUM=512, K=512):
    P[i,r,c]  = X[i,r,c] * W[c,r]                (elementwise vs W^T)
    d1[i,r]   = sum_c P[i,r,c]                   (row sums)
    d2[i,c]   = sum_r P[i,r,c]                   (col sums)
    d         = d1 + d2
    out[i,r,c] = relu(0.1*(d[i,r] + d[i,c]))

Sharding: data-parallel over the batch axis i across 8 cores (64 each).
W^T (pre-scaled by 0.1) is replicated.

Precision: X and W^T are cast to fp16 on the host (tolerance is 2e-2;
fp16 end-to-end is ~5e-4), all reductions accumulate in fp32 on device,
and the output is written as fp16 then upcast to fp32 on the host. This
halves HBM traffic vs fp32 — the kernel is DMA-bound.

Layout: the host pre-packs X per core as [NB/PAIR, 128, PAIR, NT, K]
(partition-major), so each PAIR-batch DMA is one fully contiguous
4 MB block with 32 KB contiguous runs per partition. Output uses the
same layout, inverted on the host.

Per-core engine mapping (per batch element, row r = t*128 + p):
    DMA : PAIR batches per dma_start via HWDGE (loads on SP ring,
          stores on ACT ring)
    DVE : scalar_tensor_tensor -> P chunks t=0..2 (+ d1 col accum)
    POOL: scalar_tensor_tensor -> P chunk  t=3   (+ d1 col accum)
    PE  : one accumulation group per batch in psum_gd[j]: 4 ones-matmuls
          (col sums -> d2) + 4 d1-column-stationary x identity matmuls
          (transposed d1 row), so g = d1+d2 lands complete in PSUM;
          then B = ones_row (x) g broadcast; 4 tiny k=1 matmuls -> gcol
    ACT : one [PAIR,512] g8 PSUM->SBUF copy per pair; per batch a
          [128,4] gcol copy and 4x relu(B + gcol[:,t]) -> fp16 out tile
"""

import numpy as np

N_CORES = 8
NET_SUM = 512
K = 512
NB = NET_SUM // N_CORES  # 64 batches per core
NT = 4                   # 512 rows = 4 groups of 128 partitions
P_DIM = 128

# batch elements moved per DMA (bigger -> fewer, larger transfers)
PAIR = 8
# how many of the NT product chunks run on GPSIMD (rest on DVE).
# NOTE: walrus rejects TensorScalarPtr on Pool ("Instruction engine check
# failed") — gpsimd cannot run scalar_tensor_tensor, keep this at 0.
GPS_CHUNKS = 0
# timing experiment: skip all compute, only DMA in/out
DMA_ONLY = False
# issue output DMAs from the ACT HWDGE ring (SP ring handles loads)
SPLIT_RINGS = True
# pool depths
XBUFS = 2
PBUFS = 3
# replay the whole batch loop this many times inside one NEFF (timing only)
REPEAT = 1

_CACHE = {}


def build(n_batch=NB, loop_n=1):
    from contextlib import nullcontext

    import concourse.mybir as mybir
    import concourse.tile as tile
    from concourse import bacc
    from concourse.masks import make_identity

    f32 = mybir.dt.float32
    f16 = mybir.dt.float16

    nc = bacc.Bacc("TRN2", target_bir_lowering=False, debug=False)

    pair = PAIR
    assert n_batch % pair == 0
    n_pair = n_batch // pair

    x_dram = nc.dram_tensor(
        "x4", [n_pair, P_DIM, pair, NT, K], f16, kind="ExternalInput"
    )
    wt_dram = nc.dram_tensor("wt", [P_DIM, NT, K], f16, kind="ExternalInput")
    out_dram = nc.dram_tensor(
        "out4", [n_pair, P_DIM, pair, NT, K], f16, kind="ExternalOutput"
    )

    # row-position view of a [1, 512] row AP: r = t*128 + p
    row_pat, row_kw = "o (q p) -> o q p", {"p": P_DIM}

    with tile.TileContext(nc) as tc:
        with (
            tc.tile_pool(name="const", bufs=1) as const_pool,
            tc.tile_pool(name="xp", bufs=XBUFS) as xp,
            tc.tile_pool(name="pp", bufs=PBUFS) as pp,
            tc.tile_pool(name="op", bufs=XBUFS) as op,
            tc.tile_pool(name="small", bufs=4) as small,
            tc.tile_pool(name="gdps", bufs=2, space="PSUM") as gdps,
            tc.tile_pool(name="bps", bufs=3, space="PSUM") as bps,
            tc.tile_pool(name="cps", bufs=2, space="PSUM") as cps,
        ):
            wt = const_pool.tile([P_DIM, NT, K], f16)
            nc.sync.dma_start(wt[:], wt_dram[:])

            identity = const_pool.tile([P_DIM, P_DIM], f32)
            make_identity(nc, identity[:])

            ones_col = const_pool.tile([P_DIM, 1], f16)
            nc.vector.memset(ones_col[:], 1.0)
            ones_row = const_pool.tile([1, P_DIM], f32)
            nc.vector.memset(ones_row[:], 1.0)
            one11 = const_pool.tile([1, 1], f32)
            nc.vector.memset(one11[:], 1.0)

            loop_ctx = tc.For_i(0, loop_n, 1) if loop_n > 1 else nullcontext()
            with loop_ctx:
                for ip in [i for _ in range(REPEAT) for i in range(n_pair)]:
                    # one DMA moves `pair` batch elements, fully contiguous
                    xpair = xp.tile([P_DIM, pair, NT, K], f16)
                    nc.sync.dma_start(xpair[:], x_dram[:][ip])
                    out_eng = nc.scalar if SPLIT_RINGS else nc.sync
                    if DMA_ONLY:
                        out_eng.dma_start(out_dram[:][ip], xpair[:])
                        continue
                    opair = op.tile([P_DIM, pair, NT, K], f16)

                    for j in range(pair):
                        x = xpair[:, j]
                        p = pp.tile([P_DIM, NT, K], f16)
                        d1 = small.tile([P_DIM, NT], f32, tag="d1")
                        for t in range(NT):
                            eng = nc.gpsimd if t >= NT - GPS_CHUNKS else nc.vector
                            eng.scalar_tensor_tensor(
                                out=p[:, t, :],
                                in0=x[:, t, :],
                                scalar=1.0,
                                in1=wt[:, t, :],
                                op0=mybir.AluOpType.mult,
                                op1=mybir.AluOpType.mult,
                                accum_out=d1[:, t : t + 1],
                            )
                        # g = d1 + d2 accumulates fully inside one PSUM row:
                        # 4 ones-matmuls (col sums -> d2), then 4 regular
                        # matmuls d1[:,t]^T @ I adding the transposed d1 row
                        psum_gd = gdps.tile([1, K], f32)
                        for t in range(NT):
                            nc.tensor.matmul(
                                psum_gd[:, :],
                                ones_col[:],
                                p[:, t, :],
                                start=(t == 0),
                                stop=False,
                                skip_group_check=True,
                            )
                        gd_rows = psum_gd[:].rearrange(row_pat, **row_kw)
                        for t in range(NT):
                            nc.tensor.matmul(
                                gd_rows[:, t, :],
                                d1[:, t : t + 1],
                                identity[:],
                                start=False,
                                stop=(t == NT - 1),
                                skip_group_check=True,
                            )
                        g = small.tile([1, K], f32, tag="g")
                        nc.scalar.copy(g[:], psum_gd[:])

                        # B[p,f] = G[f] for all p (rank-1 broadcast matmul)
                        psum_b = bps.tile([P_DIM, K], f32)
                        nc.tensor.matmul(
                            psum_b[:], ones_row[:], g[:], start=True, stop=True
                        )
                        # bias columns: gcol[p, t] = G[t*128 + p]
                        g_rows = g[:].rearrange(row_pat, **row_kw)
                        psum_gc = cps.tile([P_DIM, NT], f32)
                        for t in range(NT):
                            nc.tensor.matmul(
                                psum_gc[:, t : t + 1],
                                g_rows[:, t, :],
                                one11[:],
                                start=True,
                                stop=True,
                            )
                        gcol = small.tile([P_DIM, NT], f32, tag="gcol")
                        nc.scalar.copy(gcol[:], psum_gc[:])

                        # o[p, t, f] = relu(B[p,f] + gcol[p,t])
                        o = opair[:, j]
                        for t in range(NT):
                            nc.scalar.activation(
                                out=o[:, t, :],
                                in_=psum_b[:],
                                func=mybir.ActivationFunctionType.Relu,
                                bias=gcol[:, t : t + 1],
                                scale=1.0,
                            )

                    out_eng.dma_start(out_dram[:][ip], opair[:])

    nc.compile()
    return nc


def _prep_host(input_feature, kernel):
    x = np.asarray(input_feature, dtype=np.float32)
    w = np.asarray(kernel, dtype=np.float32)
    a = (0.1 * w.T).astype(np.float16)  # a[r, j] = 0.1 * w[j, r]
    wt = np.ascontiguousarray(a.reshape(NT, P_DIM, K).transpose(1, 0, 2))
    # [core, ip, p, pair, t, f] with row r = t*128 + p
    x6 = (
        x.reshape(N_CORES, NB // PAIR, PAIR, NT, P_DIM, K)
        .transpose(0, 1, 4, 2, 3, 5)
        .astype(np.float16)
    )
    return np.ascontiguousarray(x6), wt


def _unpack_out(res_list):
    # res core outputs [ip, p, pair, t, f] fp16 -> [b, r, c] fp32
    o = np.stack([r["out4"] for r in res_list], axis=0)
    o = o.transpose(0, 1, 3, 4, 2, 5).reshape(NET_SUM, K, K)
    return o.astype(np.float32)


TRACE = False
LAST_RESULTS = None


def kernel(input_feature, kernel):
    global LAST_RESULTS
    from concourse.bass_utils import run_bass_kernel_spmd

    x6, wt = _prep_host(input_feature, kernel)

    if "nc" not in _CACHE:
        _CACHE["nc"] = build()
    nc = _CACHE["nc"]

    in_maps = [{"x4": np.ascontiguousarray(x6[c]), "wt": wt} for c in range(N_CORES)]
    res = run_bass_kernel_spmd(nc, in_maps, core_ids=list(range(N_CORES)), trace=TRACE)
    LAST_RESULTS = res
    return _unpack_out(res.results)


# revision 9
# speedup vs baseline: 1.6148x; 1.2996x over previous
"""Trainium2 Bass kernel for the diag-conv problem (fp16 I/O).

Math (full problem, NET_SUM=512, K=512):
    P[i,r,c]  = X[i,r,c] * W[c,r]                (elementwise vs W^T)
    d1[i,r]   = sum_c P[i,r,c]                   (row sums)
    d2[i,c]   = sum_r P[i,r,c]                   (col sums)
    d         = d1 + d2
    out[i,r,c] = relu(0.1*(d[i,r] + d[i,c]))

Sharding: data-parallel over the batch axis i across 8 cores (64 each).
W^T (pre-scaled by 0.1) is replicated.

Precision: X and W^T are cast to fp16 on the host (tolerance is 2e-2;
fp16 end-to-end is ~5e-4), all reductions accumulate in fp32 on device,
and the output is written as fp16 then upcast to fp32 on the host. This
halves HBM traffic vs fp32 — the kernel is DMA-bound.

Layout: the host pre-packs X per core as [NB/PAIR, 128, PAIR, NT, K]
(partition-major), so each PAIR-batch DMA is one fully contiguous
4 MB block with 32 KB contiguous runs per partition. Output uses the
same layout, inverted on the host.

Per-core engine mapping (per batch element, row r = t*128 + p):
    DMA : PAIR batches per dma_start via HWDGE (loads on SP ring,
          stores on ACT ring)
    DVE : scalar_tensor_tensor -> P chunks t=0..2 (+ d1 col accum)
    POOL: scalar_tensor_tensor -> P chunk  t=3   (+ d1 col accum)
    PE  : one accumulation group per batch in psum_gd[j]: 4 ones-matmuls
          (col sums -> d2) + 4 d1-column-stationary x identity matmuls
          (transposed d1 row), so g = d1+d2 lands complete in PSUM;
          then B = ones_row (x) g broadcast; 4 tiny k=1 matmuls -> gcol
    ACT : one [PAIR,512] g8 PSUM->SBUF copy per pair; per batch a
          [128,4] gcol copy and 4x relu(B + gcol[:,t]) -> fp16 out tile
"""

import numpy as np

N_CORES = 8
NET_SUM = 512
K = 512
NB = NET_SUM // N_CORES  # 64 batches per core
NT = 4                   # 512 rows = 4 groups of 128 partitions
P_DIM = 128

# batch elements moved per DMA (bigger -> fewer, larger transfers)
PAIR = 8
# how many of the NT product chunks run on GPSIMD (rest on DVE).
# NOTE: walrus rejects TensorScalarPtr on Pool ("Instruction engine check
# failed") — gpsimd cannot run scalar_tensor_tensor, keep this at 0.
GPS_CHUNKS = 0
# timing experiment: skip all compute, only DMA in/out
DMA_ONLY = False
# timing ablation: "full" | "front" (products+sums+g copy, dummy store)
# | "back" (const-g broadcast+relu+store)
MODE = "full"
# issue output DMAs from the ACT HWDGE ring (SP ring handles loads)
SPLIT_RINGS = True
# pool depths
XBUFS = 2
PBUFS = 3
# replay the whole batch loop this many times inside one NEFF (timing only)
REPEAT = 1

_CACHE = {}


def build(n_batch=NB, loop_n=1):
    from contextlib import nullcontext

    import concourse.mybir as mybir
    import concourse.tile as tile
    from concourse import bacc
    from concourse.masks import make_identity

    f32 = mybir.dt.float32
    f16 = mybir.dt.float16

    nc = bacc.Bacc("TRN2", target_bir_lowering=False, debug=False)

    pair = PAIR
    assert n_batch % pair == 0
    n_pair = n_batch // pair

    x_dram = nc.dram_tensor(
        "x4", [n_pair, P_DIM, pair, NT, K], f16, kind="ExternalInput"
    )
    wt_dram = nc.dram_tensor("wt", [P_DIM, NT, K], f16, kind="ExternalInput")
    out_dram = nc.dram_tensor(
        "out4", [n_pair, P_DIM, pair, NT, K], f16, kind="ExternalOutput"
    )

    # row-position view of a [1, 512] row AP: r = t*128 + p
    row_pat, row_kw = "o (q p) -> o q p", {"p": P_DIM}

    with tile.TileContext(nc) as tc:
        with (
            tc.tile_pool(name="const", bufs=1) as const_pool,
            tc.tile_pool(name="xp", bufs=XBUFS) as xp,
            tc.tile_pool(name="pp", bufs=PBUFS) as pp,
            tc.tile_pool(name="op", bufs=XBUFS) as op,
            tc.tile_pool(name="small", bufs=4) as small,
            tc.tile_pool(name="gdps", bufs=2, space="PSUM") as gdps,
            tc.tile_pool(name="bps", bufs=3, space="PSUM") as bps,
            tc.tile_pool(name="cps", bufs=2, space="PSUM") as cps,
        ):
            wt = const_pool.tile([P_DIM, NT, K], f16)
            nc.sync.dma_start(wt[:], wt_dram[:])

            identity = const_pool.tile([P_DIM, P_DIM], f32)
            make_identity(nc, identity[:])

            ones_col = const_pool.tile([P_DIM, 1], f16)
            nc.vector.memset(ones_col[:], 1.0)
            ones_row = const_pool.tile([1, P_DIM], f32)
            nc.vector.memset(ones_row[:], 1.0)
            one11 = const_pool.tile([1, 1], f32)
            nc.vector.memset(one11[:], 1.0)
            gconst = const_pool.tile([1, K], f32)
            nc.vector.memset(gconst[:], 0.5)

            def _flush_back(state):
                psum_b, psum_gc, o = state
                gcol = small.tile([P_DIM, NT], f32, tag="gcol")
                nc.scalar.copy(gcol[:], psum_gc[:])
                # o[p, t, f] = relu(B[p,f] + gcol[p,t])
                for t in range(NT):
                    nc.scalar.activation(
                        out=o[:, t, :],
                        in_=psum_b[:],
                        func=mybir.ActivationFunctionType.Relu,
                        bias=gcol[:, t : t + 1],
                        scale=1.0,
                    )

            loop_ctx = tc.For_i(0, loop_n, 1) if loop_n > 1 else nullcontext()
            with loop_ctx:
                for ip in [i for _ in range(REPEAT) for i in range(n_pair)]:
                    prev = None
                    # one DMA moves `pair` batch elements, fully contiguous
                    xpair = xp.tile([P_DIM, pair, NT, K], f16)
                    nc.sync.dma_start(xpair[:], x_dram[:][ip])
                    out_eng = nc.scalar if SPLIT_RINGS else nc.sync
                    if DMA_ONLY:
                        out_eng.dma_start(out_dram[:][ip], xpair[:])
                        continue
                    opair = op.tile([P_DIM, pair, NT, K], f16)

                    for j in range(pair):
                        x = xpair[:, j]
                        if MODE != "back":
                            p = pp.tile([P_DIM, NT, K], f16)
                            d1 = small.tile([P_DIM, NT], f32, tag="d1")
                            for t in range(NT):
                                eng = (
                                    nc.gpsimd
                                    if t >= NT - GPS_CHUNKS
                                    else nc.vector
                                )
                                eng.scalar_tensor_tensor(
                                    out=p[:, t, :],
                                    in0=x[:, t, :],
                                    scalar=1.0,
                                    in1=wt[:, t, :],
                                    op0=mybir.AluOpType.mult,
                                    op1=mybir.AluOpType.mult,
                                    accum_out=d1[:, t : t + 1],
                                )
                            # g = d1 + d2 accumulates fully inside one PSUM
                            # row: 4 ones-matmuls (col sums -> d2), then 4
                            # regular matmuls d1[:,t]^T @ I adding the
                            # transposed d1 row
                            psum_gd = gdps.tile([1, K], f32)
                            for t in range(NT):
                                nc.tensor.matmul(
                                    psum_gd[:, :],
                                    ones_col[:],
                                    p[:, t, :],
                                    start=(t == 0),
                                    stop=False,
                                    skip_group_check=True,
                                )
                            gd_rows = psum_gd[:].rearrange(row_pat, **row_kw)
                            for t in range(NT):
                                nc.tensor.matmul(
                                    gd_rows[:, t, :],
                                    d1[:, t : t + 1],
                                    identity[:],
                                    start=False,
                                    stop=(t == NT - 1),
                                    skip_group_check=True,
                                )
                            g = small.tile([1, K], f32, tag="g")
                            nc.scalar.copy(g[:], psum_gd[:])
                        else:
                            g = gconst

                        if MODE == "front":
                            continue

                        # B[p,f] = G[f] for all p (rank-1 broadcast matmul)
                        psum_b = bps.tile([P_DIM, K], f32)
                        nc.tensor.matmul(
                            psum_b[:], ones_row[:], g[:], start=True, stop=True
                        )
                        # bias columns: gcol[p, t] = G[t*128 + p]
                        g_rows = g[:].rearrange(row_pat, **row_kw)
                        psum_gc = cps.tile([P_DIM, NT], f32)
                        for t in range(NT):
                            nc.tensor.matmul(
                                psum_gc[:, t : t + 1],
                                g_rows[:, t, :],
                                one11[:],
                                start=True,
                                stop=True,
                            )

                        # software-pipelined by one batch: emit batch j-1's
                        # gcol copy + relus now, so the ACT queue never
                        # stalls on this batch's PE round-trip (g -> B/gcol)
                        if prev is not None:
                            _flush_back(prev)
                        prev = (psum_b, psum_gc, opair[:, j])

                    if MODE != "front" and prev is not None:
                        _flush_back(prev)
                        prev = None

                    out_eng.dma_start(
                        out_dram[:][ip],
                        xpair[:] if MODE == "front" else opair[:],
                    )

    nc.compile()
    return nc


def _prep_host(input_feature, kernel):
    x = np.asarray(input_feature, dtype=np.float32)
    w = np.asarray(kernel, dtype=np.float32)
    a = (0.1 * w.T).astype(np.float16)  # a[r, j] = 0.1 * w[j, r]
    wt = np.ascontiguousarray(a.reshape(NT, P_DIM, K).transpose(1, 0, 2))
    # [core, ip, p, pair, t, f] with row r = t*128 + p
    x6 = (
        x.reshape(N_CORES, NB // PAIR, PAIR, NT, P_DIM, K)
        .transpose(0, 1, 4, 2, 3, 5)
        .astype(np.float16)
    )
    return np.ascontiguousarray(x6), wt


def _unpack_out(res_list):
    # res core outputs [ip, p, pair, t, f] fp16 -> [b, r, c] fp32
    o = np.stack([r["out4"] for r in res_list], axis=0)
    o = o.transpose(0, 1, 3, 4, 2, 5).reshape(NET_SUM, K, K)
    return o.astype(np.float32)


TRACE = False
LAST_RESULTS = None


def kernel(input_feature, kernel):
    global LAST_RESULTS
    from concourse.bass_utils import run_bass_kernel_spmd

    x6, wt = _prep_host(input_feature, kernel)

    if "nc" not in _CACHE:
        _CACHE["nc"] = build()
    nc = _CACHE["nc"]

    in_maps = [{"x4": np.ascontiguousarray(x6[c]), "wt": wt} for c in range(N_CORES)]
    res = run_bass_kernel_spmd(nc, in_maps, core_ids=list(range(N_CORES)), trace=TRACE)
    LAST_RESULTS = res
    return _unpack_out(res.results)


# revision 12
# speedup vs baseline: 2.2228x; 1.3766x over previous
"""Trainium2 Bass kernel for the diag-conv problem (fp16 I/O).

Math (full problem, NET_SUM=512, K=512):
    P[i,r,c]  = X[i,r,c] * W[c,r]                (elementwise vs W^T)
    d1[i,r]   = sum_c P[i,r,c]                   (row sums)
    d2[i,c]   = sum_r P[i,r,c]                   (col sums)
    d         = d1 + d2
    out[i,r,c] = relu(0.1*(d[i,r] + d[i,c]))

Sharding: data-parallel over the batch axis i across 8 cores (64 each).
W^T (pre-scaled by 0.1) is replicated.

Precision: X and W^T are cast to fp16 on the host (tolerance is 2e-2;
fp16 end-to-end is ~5e-4), all reductions accumulate in fp32 on device,
and the output is written as fp16 then upcast to fp32 on the host. This
halves HBM traffic vs fp32 — the kernel is DMA-bound.

Layout: the host pre-packs X per core as [NB/PAIR, 128, PAIR, NT, K]
(partition-major), so each PAIR-batch DMA is one fully contiguous
4 MB block with 32 KB contiguous runs per partition. Output uses the
same layout, inverted on the host.

Engine mapping per batch (row r = t*128 + p). The d-row NEVER
materializes as a [1,512] SBUF row — both broadcast forms are built
directly in PSUM by the PE, which kills the ACT<->PE g-copy round-trip
that serialized earlier versions:
    DMA : PAIR batches per dma_start via HWDGE (loads on SP ring,
          stores on ACT ring)
    DVE : scalar_tensor_tensor -> P (+ d1 column accum), 4 chunks.
          STT has no 2x uop mode (1 elem/lane/cycle): ~2.8us/batch,
          the pipeline cap alongside DMA.
    PE  : bc[p,f] = d[f] for ALL p, in one PSUM accumulation group:
          4 ones-stationary matmuls (col sums of P, broadcast) + 4
          stride-0-stationary matmuls (d1[:,t] bcast x identity).
          gc[p,t] = d2[t*128+p] via 16 P-slice-stationary x ones_col
          matmuls (4 accumulating per t).
    ACT : one [128,4] copy (gc -> SBUF) + 4 relu(bc + gcol[:,t]) per
          batch (ACT fixed cost ~220 cycles/instr favors few big ops).
    POOL: gcol = gc_sbuf + d1 ([128,4] SBUF add - GPSIMD cannot touch
          PSUM, so the copy above feeds it).

The back stage (gc copy / add / relus) is software-pipelined two
batches behind the front so the ACT queue never stalls on PE or POOL
round-trips.
"""

import numpy as np

N_CORES = 8
NET_SUM = 512
K = 512
NB = NET_SUM // N_CORES  # 64 batches per core
NT = 4                   # 512 rows = 4 groups of 128 partitions
P_DIM = 128

# batch elements moved per DMA (bigger -> fewer, larger transfers)
PAIR = 8
# timing experiment: skip all compute, only DMA in/out
DMA_ONLY = False
# timing ablation: "full" | "front" (products+sums, dummy store)
# | "back" (broadcast+relu+store from memset PSUM)
MODE = "full"
# issue output DMAs from the ACT HWDGE ring (SP ring handles loads)
SPLIT_RINGS = True
# do the gcol = gc + d1 add on gpsimd (True) or vector (False)
POOL_ADD = True
# pool depths
XBUFS = 2
PBUFS = 3
BCBUFS = 3
# replay the whole batch loop this many times inside one NEFF (timing only)
REPEAT = 1

_CACHE = {}


def build(n_batch=NB, loop_n=1):
    from contextlib import nullcontext

    import concourse.mybir as mybir
    import concourse.tile as tile
    from concourse import bacc
    from concourse.masks import make_identity

    f32 = mybir.dt.float32
    f16 = mybir.dt.float16

    nc = bacc.Bacc("TRN2", target_bir_lowering=False, debug=False)

    pair = PAIR
    assert n_batch % pair == 0
    n_pair = n_batch // pair

    x_dram = nc.dram_tensor(
        "x4", [n_pair, P_DIM, pair, NT, K], f16, kind="ExternalInput"
    )
    wt_dram = nc.dram_tensor("wt", [P_DIM, NT, K], f16, kind="ExternalInput")
    out_dram = nc.dram_tensor(
        "out4", [n_pair, P_DIM, pair, NT, K], f16, kind="ExternalOutput"
    )

    with tile.TileContext(nc) as tc:
        with (
            tc.tile_pool(name="const", bufs=1) as const_pool,
            tc.tile_pool(name="xp", bufs=XBUFS) as xp,
            tc.tile_pool(name="pp", bufs=PBUFS) as pp,
            tc.tile_pool(name="op", bufs=XBUFS) as op,
            tc.tile_pool(name="small", bufs=4) as small,
            tc.tile_pool(name="bcps", bufs=BCBUFS, space="PSUM") as bcps,
            tc.tile_pool(name="gcps", bufs=3, space="PSUM") as gcps,
        ):
            wt = const_pool.tile([P_DIM, NT, K], f16)
            nc.sync.dma_start(wt[:], wt_dram[:])

            identity = const_pool.tile([P_DIM, P_DIM], f32)
            make_identity(nc, identity[:])

            ones_col = const_pool.tile([P_DIM, 1], f16)
            nc.vector.memset(ones_col[:], 1.0)
            ones_sq = const_pool.tile([P_DIM, P_DIM], f16)
            nc.vector.memset(ones_sq[:], 1.0)

            add_eng = nc.gpsimd if POOL_ADD else nc.vector

            def _flush_back(state):
                # stage 2 of the back half for one batch: SBUF add then the
                # four output relus (bias = gcol column, input = bc bank)
                psum_bc, gc_s, d1, o = state
                gcol = small.tile([P_DIM, NT], f32, tag="gcol")
                add_eng.tensor_tensor(
                    out=gcol[:], in0=gc_s[:], in1=d1[:], op=mybir.AluOpType.add
                )
                for t in range(NT):
                    nc.scalar.activation(
                        out=o[:, t, :],
                        in_=psum_bc[:],
                        func=mybir.ActivationFunctionType.Relu,
                        bias=gcol[:, t : t + 1],
                        scale=1.0,
                    )

            loop_ctx = tc.For_i(0, loop_n, 1) if loop_n > 1 else nullcontext()
            with loop_ctx:
                for ip in [i for _ in range(REPEAT) for i in range(n_pair)]:
                    pending = []
                    # one DMA moves `pair` batch elements, fully contiguous
                    xpair = xp.tile([P_DIM, pair, NT, K], f16)
                    nc.sync.dma_start(xpair[:], x_dram[:][ip])
                    out_eng = nc.scalar if SPLIT_RINGS else nc.sync
                    if DMA_ONLY:
                        out_eng.dma_start(out_dram[:][ip], xpair[:])
                        continue
                    opair = op.tile([P_DIM, pair, NT, K], f16)

                    for j in range(pair):
                        x = xpair[:, j]
                        p = pp.tile([P_DIM, NT, K], f16)
                        d1 = small.tile([P_DIM, NT], f32, tag="d1")
                        psum_bc = bcps.tile([P_DIM, K], f32)
                        psum_gc = gcps.tile([P_DIM, NT], f32)
                        if MODE != "back":
                            for t in range(NT):
                                nc.vector.scalar_tensor_tensor(
                                    out=p[:, t, :],
                                    in0=x[:, t, :],
                                    scalar=1.0,
                                    in1=wt[:, t, :],
                                    op0=mybir.AluOpType.mult,
                                    op1=mybir.AluOpType.mult,
                                    accum_out=d1[:, t : t + 1],
                                )
                            # bc[p,f] = d2[f] + d1[f] for every p, one PSUM
                            # accumulation group: ones-stationary col-sum
                            # broadcasts + stride-0-stationary d1 broadcasts
                            for t in range(NT):
                                nc.tensor.matmul(
                                    psum_bc[:, :],
                                    ones_sq[:],
                                    p[:, t, :],
                                    start=(t == 0),
                                    stop=False,
                                    skip_group_check=True,
                                )
                            for t in range(NT):
                                nc.tensor.matmul(
                                    psum_bc[:, t * P_DIM : (t + 1) * P_DIM],
                                    d1[:, t : t + 1].broadcast_to(
                                        [P_DIM, P_DIM]
                                    ),
                                    identity[:],
                                    start=False,
                                    stop=(t == NT - 1),
                                    skip_group_check=True,
                                )
                            # gc[p,t] = d2[t*128+p]: P-slice-stationary sums
                            for t in range(NT):
                                for q in range(NT):
                                    nc.tensor.matmul(
                                        psum_gc[:, t : t + 1],
                                        p[:, q, t * P_DIM : (t + 1) * P_DIM],
                                        ones_col[:],
                                        start=(q == 0),
                                        stop=(q == NT - 1),
                                        skip_group_check=True,
                                    )
                        else:
                            nc.vector.memset(d1[:], 0.1)

                        if MODE == "front":
                            # consume psum tiles so pools rotate
                            junk = small.tile([P_DIM, NT], f32, tag="gcol")
                            nc.scalar.copy(junk[:], psum_gc[:])
                            continue

                        # back stage 1: gc -> SBUF (gpsimd can't read PSUM)
                        gc_s = small.tile([P_DIM, NT], f32, tag="gc")
                        nc.scalar.copy(gc_s[:], psum_gc[:])
                        pending.append((psum_bc, gc_s, d1, opair[:, j]))
                        # stage 2 runs two batches behind: the ACT->POOL->ACT
                        # chain of batch j-2 is fully latency-hidden
                        if len(pending) > 2:
                            _flush_back(pending.pop(0))

                    for state in pending:
                        _flush_back(state)

                    out_eng.dma_start(
                        out_dram[:][ip],
                        xpair[:] if MODE == "front" else opair[:],
                    )

    nc.compile()
    return nc


def _prep_host(input_feature, kernel):
    x = np.asarray(input_feature, dtype=np.float32)
    w = np.asarray(kernel, dtype=np.float32)
    a = (0.1 * w.T).astype(np.float16)  # a[r, j] = 0.1 * w[j, r]
    wt = np.ascontiguousarray(a.reshape(NT, P_DIM, K).transpose(1, 0, 2))
    # [core, ip, p, pair, t, f] with row r = t*128 + p
    x6 = (
        x.reshape(N_CORES, NB // PAIR, PAIR, NT, P_DIM, K)
        .transpose(0, 1, 4, 2, 3, 5)
        .astype(np.float16)
    )
    return np.ascontiguousarray(x6), wt


def _unpack_out(res_list):
    # per-core outputs [ip, p, pair, t, f] fp16 -> [b, r, c] fp32
    o = np.stack([r["out4"] for r in res_list], axis=0)
    o = o.transpose(0, 1, 3, 4, 2, 5).reshape(NET_SUM, K, K)
    return o.astype(np.float32)


TRACE = False
LAST_RESULTS = None


def kernel(input_feature, kernel):
    global LAST_RESULTS
    from concourse.bass_utils import run_bass_kernel_spmd

    x6, wt = _prep_host(input_feature, kernel)

    if "nc" not in _CACHE:
        _CACHE["nc"] = build()
    nc = _CACHE["nc"]

    in_maps = [{"x4": np.ascontiguousarray(x6[c]), "wt": wt} for c in range(N_CORES)]
    res = run_bass_kernel_spmd(nc, in_maps, core_ids=list(range(N_CORES)), trace=TRACE)
    LAST_RESULTS = res
    return _unpack_out(res.results)
